# revision 64
# baseline (speedup 1.0000x reference)
"""Trainium2 Bass kernel for nn_Slots: out[b,s,d] = sum_hw feats[b,d,hw] * masks[s,hw].

Fast path (data-parallel over B across 8 cores, 32 batches/core) exploits the
9-rectangle structure of the 9-choose-4 masks: every mask is constant on each
cell of a 3x3 rectangle grid, so

    out[b,s,d] = sum_k M2[s,k] * colsum[b,d,k],   k = (w-band, h), 84 values,

where colsum[b,d,(j2,h)] = sum_{w in band j2} feats[b,d,h,w] and M2 is a tiny
(126, 84) matrix computed on the host from the masks. Per batch:

  - HWDGE load feats[b] (512, 784) fp32 -> nat (SP queue, 4-deep rotation)
  - 3 DVE tensor_reduce (one per w-band, all 4 d-blocks): nat -> colsum
  - 4 PE transposes: colsum -> pT psum [84, 512]
  - ACT copy pT -> p84 sbuf (f32r)
  - 1 PE matmul: po [126, 512] = M2T.T @ p84  (contraction over 84)
  - ACT copy po -> ot fp16; HWDGE store ot -> out[b] (ACT queue)

All DMA goes through HWDGE (sync/scalar engines), so descriptor emission
overlaps data movement and the 16-engine DMA device stays ~100% busy: the
kernel is bound by the irreducible HBM traffic (51.4 MB feats in + 4.1 MB
fp16 out per core at the cost model's 360 GB/s). The last four batches run a
finer-grained pipeline (per-j-block chunked loads and reduces, the final
chunk h-split and transposed into its own PSUM tile, the final matmul and
out-copy split in halves across PE/DVE/ACT, both final half-stores issued
from the idle SP queue) to shrink the drain tail; the kernel-tail drain
keeps the last store's completion-sem wait on itself so its other split
waits process while that sem is still pending.

Output is computed/stored as fp16 (error ~5e-4 << the 2e-2 gate) and upcast
to fp32 on the host.

If the masks ever fail the exact rectangle-decomposition check, kernel()
falls back to the previous general einsum kernel (PE-transpose pipeline,
kept verbatim below).
"""

import numpy as np
from contextlib import ExitStack

import concourse.bass as bass
import concourse.tile as tile
import concourse.tile_sem_assignment as _tsa
from concourse import mybir
from concourse.bass_utils import run_bass_kernel_spmd
from concourse.tile_rust import add_dep_helper

_tsa.NUM_SWDGE_GLOBAL_SEMS = 8

N_CORES = 8
B_FULL, D, H, W = 256, 512, 28, 28
HW = H * W           # 784
S = 126
B_LOC = B_FULL // N_CORES  # 32
NJ = D // 128        # 4 d-blocks of 128 per batch
K84 = 84             # 3 w-bands x 28 rows
GRID = [0, 9, 19, 28]  # round(i*28/3)
BANDS = [(GRID[i], GRID[i + 1]) for i in range(3)]

F32 = mybir.dt.float32
F32R = mybir.dt.float32r
F16 = mybir.dt.float16

_CACHE = {}
SPLIT_DRAIN = True  # set False for CoreSim (it rejects post-scheduler NoOps)
PREFETCH = 3        # loads run up to 3 batches ahead (nat rotation is 4)


def _build_program():
    nc = bass.Bass("TRN2", target_bir_lowering=False, debug=False)
    feats = nc.dram_tensor("feats", (B_LOC, D, HW), F32, kind="ExternalInput").ap()
    m2t = nc.dram_tensor("m2t", (K84, S), F32, kind="ExternalInput").ap()
    out = nc.dram_tensor("out", (B_LOC, S, D), F16, kind="ExternalOutput").ap()

    with ExitStack() as ctx:
        tc = ctx.enter_context(tile.TileContext(nc))
        const_pool = ctx.enter_context(tc.tile_pool(name="const", bufs=1))
        nat_pool = ctx.enter_context(tc.tile_pool(name="nat", bufs=1))
        cs_pool = ctx.enter_context(tc.tile_pool(name="cs", bufs=1))
        p84_pool = ctx.enter_context(tc.tile_pool(name="p84", bufs=1))
        ot_pool = ctx.enter_context(tc.tile_pool(name="ot", bufs=1))
        pt_pool = ctx.enter_context(tc.tile_pool(name="ptp", bufs=1, space="PSUM"))
        po_pool = ctx.enter_context(tc.tile_pool(name="pop", bufs=1, space="PSUM"))
        scr_pool = ctx.enter_context(tc.tile_pool(name="scrp", bufs=1, space="PSUM"))

        # identity built on gpsimd (Pool queue, overlaps the first loads)
        ones_t = const_pool.tile([128, 128], F32, name="ones_t")
        nc.gpsimd.memset(ones_t[:], 1.0)
        id_t = const_pool.tile([128, 128], F32, name="id_t")
        nc.gpsimd.affine_select(
            id_t[:], ones_t[:], pattern=[[1, 128]],
            compare_op=mybir.AluOpType.is_equal, fill=0.0,
            base=0, channel_multiplier=-1,
        )

        # feats loads go first on the SP queue so DMA starts immediately.
        # Each batch is loaded as 4 per-j-block chunk DMAs so the DVE
        # reduces can trail the load by one chunk (shrinks the drain tail).
        nats = [nat_pool.tile([128, NJ * HW], F32, name=f"nat{i}",
                              tag=f"nat{i}", bufs=1) for i in range(4)]

        def load(b):
            natv = nats[b % 4].rearrange("p (j q) -> p j q", q=HW)
            fv = feats[b].rearrange("(j p) q -> p j q", p=128)
            if b < B_LOC - 4:
                nc.sync.dma_start(natv, fv)
                return
            # last four batches: per-j chunks so the drain tail can track the
            # load at chunk granularity; the very last chunk is h-split so
            # the final reduces are half-size
            nath = nats[b % 4].rearrange("p (j h w) -> p j h w", h=H, w=W)
            fh = feats[b].rearrange("(j p) (h w) -> p j h w", p=128, w=W)
            for j in range(NJ):
                if b == B_LOC - 1 and j == NJ - 1:
                    nc.sync.dma_start(nath[:, j:j + 1, 0:14], fh[:, j:j + 1, 0:14])
                    nc.sync.dma_start(nath[:, j:j + 1, 14:H], fh[:, j:j + 1, 14:H])
                else:
                    nc.sync.dma_start(natv[:, j:j + 1], fv[:, j:j + 1])

        for b in range(PREFETCH):
            load(b)

        m2_t = const_pool.tile([K84, S], F32, name="m2_t")
        nc.sync.dma_start(m2_t[:], m2t)
        m2_r = const_pool.tile([K84, S], F32R, name="m2_r")
        nc.vector.tensor_copy(m2_r[:], m2_t[:])

        # warm-up fence: first PE op; absorbs the gpsimd tick for id_t
        scr = scr_pool.tile([128, 128], F32, name="scr")
        warm0 = nc.tensor.matmul(scr[0:2, :], id_t[:, 0:2], id_t[:],
                                 start=True, stop=True, is_transpose=True)

        css = [cs_pool.tile([128, NJ * K84], F32, name=f"cs{i}",
                            tag=f"cs{i}", bufs=1) for i in range(2)]
        pt_last = pt_pool.tile([K84, 128], F32, name="pt_last",
                               tag="pt_last", bufs=1)
        p84s = [p84_pool.tile([K84, D], F32R, name=f"p84{i}",
                              tag=f"p84{i}", bufs=1) for i in range(2)]
        ots = [ot_pool.tile([S, D], F16, name=f"ot{i}",
                            tag=f"ot{i}", bufs=1) for i in range(4)]
        # the last four batches get their own staging tiles so the deferred
        # stores' slots are never rewritten (Tile deps follow emission order)
        ot_ded = {B_LOC - 4 + i: ot_pool.tile([S, D], F16, name=f"otd{i}",
                                              tag=f"otd{i}", bufs=1)
                  for i in range(4)}

        prev_pe = warm0
        deferred = []   # (b, ot) stores pulled out of the final load stream
        gate31 = None   # first j3 reduce of the last batch (fires once the
                        # last load chunk has landed)
        for b in range(B_LOC):
            if b + PREFETCH < B_LOC:
                load(b + PREFETCH)
            nat = nats[b % 4]
            cs = css[b % 2]
            pt = pt_pool.tile([K84, D], F32, name="pt", tag=f"pt{b % 2}", bufs=1)
            po = po_pool.tile([S, D], F32, name="po", tag=f"po{b % 2}", bufs=1)
            p84 = p84s[b % 2]
            ot = ot_ded.get(b, ots[b % 4])
            natv = nat.rearrange("p (j h w) -> p j h w", h=H, w=W)
            csv = cs.rearrange("p (j k) -> p j k", k=K84)

            if b < B_LOC - 4:
                # one DVE reduce per w-band covering all 4 j-blocks
                for bi, (w0, w1) in enumerate(BANDS):
                    nc.vector.tensor_reduce(
                        csv[:, :, bi * H:(bi + 1) * H],
                        natv[:, :, :, w0:w1],
                        axis=mybir.AxisListType.X, op=mybir.AluOpType.add,
                    )
                for j in range(NJ):
                    t = nc.tensor.matmul(
                        pt[:, j * 128:(j + 1) * 128],
                        cs[:, j * K84:(j + 1) * K84], id_t[:],
                        start=True, stop=True, is_transpose=True,
                    )
                    add_dep_helper(t.ins, prev_pe.ins, sync=False, reason="order")
                    prev_pe = t
                nc.scalar.activation(p84[:], pt[:],
                                     mybir.ActivationFunctionType.Copy)
            else:
                # fine-grained per-j pipeline for the drain tail: 3 DVE
                # reduces then a PE transpose per chunk. The very last chunk
                # is h-split (matching the load) and transposed into its own
                # PSUM tile so its copy is not serialized behind the others.
                last = (b == B_LOC - 1)
                for j in range(NJ):
                    hsplits = [(0, 14), (14, H)] if (last and j == NJ - 1) \
                        else [(0, H)]
                    for h0, h1 in hsplits:
                        for bi, (w0, w1) in enumerate(BANDS):
                            rd = nc.vector.tensor_reduce(
                                csv[:, j:j + 1, bi * H + h0: bi * H + h1],
                                natv[:, j:j + 1, h0:h1, w0:w1],
                                axis=mybir.AxisListType.X,
                                op=mybir.AluOpType.add,
                            )
                            if last and j == 2 and gate31 is None:
                                gate31 = rd
                    dst = (pt_last if (last and j == NJ - 1)
                           else pt[:, j * 128:(j + 1) * 128])
                    t = nc.tensor.matmul(
                        dst[:, 0:128] if (last and j == NJ - 1) else dst,
                        cs[:, j * K84:(j + 1) * K84], id_t[:],
                        start=True, stop=True, is_transpose=True,
                    )
                    add_dep_helper(t.ins, prev_pe.ins, sync=False, reason="order")
                    prev_pe = t
                    if last and j == NJ - 2:
                        nc.scalar.activation(
                            p84[:, 0:(NJ - 1) * 128], pt[:, 0:(NJ - 1) * 128],
                            mybir.ActivationFunctionType.Copy)
                        # first-half matmul + DVE out-copy run while the j3
                        # chunk is still in flight
                        mm_a = nc.tensor.matmul(
                            po[:, 0:D // 2], m2_r[:], p84[:, 0:D // 2],
                            start=True, stop=True)
                        add_dep_helper(mm_a.ins, prev_pe.ins, sync=False,
                                       reason="order")
                        prev_pe = mm_a
                        nc.vector.tensor_copy(ot[:, 0:D // 2],
                                              po[:, 0:D // 2])
                if last:
                    nc.scalar.activation(
                        p84[:, (NJ - 1) * 128:], pt_last[:],
                        mybir.ActivationFunctionType.Copy)
                else:
                    nc.scalar.activation(p84[:], pt[:],
                                         mybir.ActivationFunctionType.Copy)

            # stage 3: po [126, 512] = m2_r.T @ p84 (contraction over 84)
            if b < B_LOC - 1:
                mm = nc.tensor.matmul(po[:], m2_r[:], p84[:],
                                      start=True, stop=True)
                add_dep_helper(mm.ins, prev_pe.ins, sync=False, reason="order")
                prev_pe = mm
                nc.scalar.activation(ot[:], po[:],
                                     mybir.ActivationFunctionType.Copy)
            else:
                # second-half matmul + ACT out-copy (first half already done)
                mm = nc.tensor.matmul(po[:, D // 2:], m2_r[:], p84[:, D // 2:],
                                      start=True, stop=True)
                add_dep_helper(mm.ins, prev_pe.ins, sync=False, reason="order")
                prev_pe = mm
                nc.scalar.activation(ot[:, D // 2:], po[:, D // 2:],
                                     mybir.ActivationFunctionType.Copy)
            if b < B_LOC - 1:
                if b in (B_LOC - 6, B_LOC - 5, B_LOC - 4):
                    # these stores would otherwise interleave inside the
                    # final load stream, delaying the DVE reduce spine that
                    # bounds the drain tail; defer them into the tail's idle
                    # DMA window instead (all load requests are already
                    # queued at the DMA device by then)
                    deferred.append((b, ot))
                else:
                    nc.scalar.dma_start(out[b], ot[:])
            else:
                # deferred stores: first few via HWDGE on SP (cheap emission,
                # well before the critical h0/h1 emissions), the rest via
                # SWDGE on the otherwise-idle Pool queue
                for bb, ott in deferred:
                    st = nc.gpsimd.dma_start(out[bb], ott[:])
                    add_dep_helper(st.ins, gate31.ins, sync=True,
                                   reason="defer-store")
                # first half was copied by DVE while j3 was in flight; store
                # it from the idle SP queue so only the second half's store
                # trails the final out-copy
                nc.sync.dma_start(out[b][:, 0:D // 2], ot[:, 0:D // 2])
                nc.sync.dma_start(out[b][:, D // 2:], ot[:, D // 2:])

    if SPLIT_DRAIN:
        _split_drain_waits(nc)
    return nc


def _split_drain_waits(nc, max_waits=1):
    """TRN2 queue instructions support one sync wait. Anything the scheduler
    left with more gets its excess waits moved onto single-wait NoOps inserted
    right before it on the same engine queue (in-order, so the semantics are
    identical). For the kernel-tail drains, the wait on the LAST DMA's
    completion sem (the one that fires last) is kept on the drain itself so
    the other waits' NoOps process while it is still pending."""
    for f in nc.m.functions:
        last_dma_sem = None
        for blk in getattr(f, "blocks", []):
            for inst in blk.instructions:
                if isinstance(inst, mybir.InstDMACopy):
                    si = getattr(inst, "sync_info", None)
                    if si is not None and si.on_update:
                        last_dma_sem = si.on_update[0].id
        for blk in getattr(f, "blocks", []):
            insts = blk.instructions
            i = 0
            while i < len(insts):
                inst = insts[i]
                si = getattr(inst, "sync_info", None)
                if (si is not None and len(si.on_wait) > max_waits):
                    waits = list(si.on_wait)
                    if isinstance(inst, mybir.InstDrain) and last_dma_sem is not None:
                        waits.sort(key=lambda w: w.id == last_dma_sem)
                    keep = waits[-max_waits:]
                    move = waits[:-max_waits]
                    for k, w in enumerate(move):
                        nop = mybir.InstNoOp(
                            name=f"{inst.name}-ws{k}",
                            engine=inst.engine,
                            bass_nofuse=True,
                            sync_info=mybir.SyncInfo(on_wait=[w], on_update=[]),
                        )
                        insts.insert(i, nop)
                        i += 1
                    si.on_wait = keep
                i += 1


def get_program():
    if "nc" not in _CACHE:
        _CACHE["nc"] = _build_program()
    return _CACHE["nc"]


def _decompose_masks(masks):
    """Return M2T (84, 126) float32 if masks are exactly constant on the 3x3
    rectangle grid (the 9c4 slot masks are), else None."""
    masks = np.asarray(masks, dtype=np.float32)
    if masks.shape != (S, H, W):
        return None
    R = np.zeros((S, 9), dtype=np.float32)
    rec = np.zeros_like(masks)
    for r in range(9):
        i, j = divmod(r, 3)
        y0, y1 = GRID[i], GRID[i + 1]
        x0, x1 = GRID[j], GRID[j + 1]
        R[:, r] = masks[:, y0, x0]
        rec[:, y0:y1, x0:x1] = R[:, r][:, None, None]
    if not np.allclose(rec, masks, rtol=0, atol=1e-7):
        return None
    # M2[s, k], k = band*28 + h; band = w-band index, h = row
    M2 = np.zeros((S, K84), dtype=np.float32)
    for bi in range(3):
        for h in range(H):
            i = 0 if h < GRID[1] else (1 if h < GRID[2] else 2)
            M2[:, bi * H + h] = R[:, i * 3 + bi]
    return np.ascontiguousarray(M2.T)


def make_in_maps(feats, masks):
    feats = np.ascontiguousarray(np.asarray(feats, dtype=np.float32))
    fr = feats.reshape(N_CORES, B_LOC, D, HW)
    m2t = _decompose_masks(masks)
    assert m2t is not None
    return [{"feats": fr[i], "m2t": m2t} for i in range(N_CORES)]


def kernel(feats, masks, _trace=False, _tmpdir=None):
    if _decompose_masks(masks) is None:
        return _kernel_fb(feats, masks, _trace=_trace, _tmpdir=_tmpdir)
    nc = get_program()
    in_maps = make_in_maps(feats, masks)
    res = run_bass_kernel_spmd(
        nc, in_maps, core_ids=list(range(N_CORES)),
        trace=_trace, tmpdir=_tmpdir,
    )
    out = np.concatenate([r["out"] for r in res.results], axis=0)
    if _trace:
        _CACHE["last_results"] = res
    return out.astype(np.float32)


# ---------------------------------------------------------------------------
# Fallback: general einsum kernel (previous implementation, used only if the
# masks are not exactly rectangle-decomposable).
# ---------------------------------------------------------------------------

KC = 112             # hw contraction chunk (7 * 112 = 784)
NCHUNK = HW // KC    # 7
USE_F32R_MM = True


def _build_program_fb():
    nc = bass.Bass("TRN2", target_bir_lowering=False, debug=False)
    feats = nc.dram_tensor("feats", (B_LOC, D, HW), F32, kind="ExternalInput").ap()
    masksT = nc.dram_tensor("masksT", (HW, S), F32, kind="ExternalInput").ap()
    out = nc.dram_tensor("out", (B_LOC, S, D), F32, kind="ExternalOutput").ap()

    with ExitStack() as ctx:
        tc = ctx.enter_context(tile.TileContext(nc))
        const_pool = ctx.enter_context(tc.tile_pool(name="const", bufs=1))
        nat_pool = ctx.enter_context(tc.tile_pool(name="nat", bufs=1))
        ft_pool = ctx.enter_context(tc.tile_pool(name="ftp", bufs=2))
        ot_pool = ctx.enter_context(tc.tile_pool(name="otp", bufs=1))
        pt_pool = ctx.enter_context(tc.tile_pool(name="ptp", bufs=1, space="PSUM"))
        po_pool = ctx.enter_context(tc.tile_pool(name="pop", bufs=1, space="PSUM"))
        scr_pool = ctx.enter_context(tc.tile_pool(name="scrp", bufs=1, space="PSUM"))

        def order(later, earlier):
            add_dep_helper(later.ins, earlier.ins, sync=False, reason="order")

        ones_t = const_pool.tile([128, 128], F32, name="ones_t")
        nc.gpsimd.memset(ones_t[:], 1.0)
        id_t = const_pool.tile([128, 128], F32, name="id_t")
        nc.gpsimd.affine_select(
            id_t[:], ones_t[:], pattern=[[1, 128]],
            compare_op=mybir.AluOpType.is_equal, fill=0.0,
            base=0, channel_multiplier=-1,
        )

        mk_t = const_pool.tile([KC, NCHUNK * S], F32, name="mk_t")
        nc.sync.dma_start(
            mk_t.rearrange("p (c s) -> p c s", s=S),
            masksT.rearrange("(c p) s -> p c s", p=KC),
        )
        if USE_F32R_MM:
            mk_r = const_pool.tile([KC, NCHUNK * S], F32R, name="mk_r")
            nc.vector.tensor_copy(mk_r[:], mk_t[:])
        else:
            mk_r = mk_t

        scr = scr_pool.tile([128, 128], F32, name="scr")
        rscr = const_pool.tile([1, 8], F32, name="rscr")
        rscr_act = const_pool.tile([1, 8], F32, name="rscr_act")
        pscr = const_pool.tile([1, 8], F32, name="pscr")
        pa = [const_pool.tile([1, 8], F32, name=f"pa{i}", tag=f"pa{i}", bufs=1)
              for i in range(2)]
        pb = [const_pool.tile([1, 8], F32, name=f"pb{i}", tag=f"pb{i}", bufs=1)
              for i in range(2)]
        pscr2 = const_pool.tile([1, 8], F32, name="pscr2")

        warm0 = nc.tensor.matmul(scr[0:2, :], id_t[:, 0:2], id_t[:],
                                 start=True, stop=True, is_transpose=True)

        nats = []
        ots = []
        prev_pe = warm0
        prev_dve = None
        prev_act = None
        prev_pool = None

        def flush_out(bb):
            nonlocal prev_pool, prev_act
            ot = ots[bb]
            a3 = nc.gpsimd.tensor_copy(pb[bb % 2][0:1, 0:4], ot[0:1, 0:4])
            if prev_pool is not None:
                order(a3, prev_pool)
            a4 = nc.gpsimd.tensor_copy(pscr2[0:1, 0:4], pb[bb % 2][0:1, 0:4])
            order(a4, a3)
            dma_out = nc.gpsimd.dma_start(out[bb % B_LOC], ot[:])
            order(dma_out, a4)
            prev_pool = dma_out

        for gb in range(B_LOC):
            b = gb
            nat = nat_pool.tile([128, NJ * HW], F32, name="nat",
                                tag=f"nat{gb % 4}", bufs=1)
            if gb >= 3:
                a1 = nc.gpsimd.tensor_copy(pa[gb % 2][0:1, 0:4],
                                           nats[gb - 3][0:1, 0:4])
                if prev_pool is not None:
                    order(a1, prev_pool)
                a2 = nc.gpsimd.tensor_copy(pscr[0:1, 0:4], pa[b % 2][0:1, 0:4])
                order(a2, a1)
                prev_pool = a2
            dma_in = nc.gpsimd.dma_start(
                nat.rearrange("p (j q) -> p j q", q=HW),
                feats[b].rearrange("(j p) q -> p j q", p=128),
            )
            if prev_pool is not None:
                order(dma_in, prev_pool)
            prev_pool = dma_in
            nats.append(nat)
            if gb >= 2:
                flush_out(gb - 2)

            fence = nc.tensor.matmul(scr[0:2, :], nat[:, 0:2], id_t[:],
                                     start=True, stop=True, is_transpose=True)
            order(fence, prev_pe)
            prev_pe = fence

            if gb >= 2:
                pf = nc.tensor.matmul(scr[0:2, 0:126], ots[gb - 2][:, 0:2],
                                      id_t[0:126, 0:126],
                                      start=True, stop=True, is_transpose=True)
                order(pf, prev_pe)
                prev_pe = pf

            fts = []
            po = po_pool.tile([S, D], F32, name="po", tag=f"po{gb % 2}", bufs=1)

            def emit_mm(c):
                nonlocal prev_pe
                mm = nc.tensor.matmul(
                    po[:], mk_r[:, c * S:(c + 1) * S], fts[c][:],
                    start=(c == 0), stop=(c == NCHUNK - 1),
                )
                order(mm, prev_pe)
                prev_pe = mm
                return mm

            for c in range(NCHUNK):
                if c >= 3:
                    emit_mm(c - 3)
                pt = pt_pool.tile([KC, NJ * 128], F32, name="pt",
                                  tag=f"pt{c % 3}", bufs=1)
                for j in range(NJ):
                    src = nat[:, j * HW + c * KC: j * HW + (c + 1) * KC]
                    dst = pt[:, j * 128:(j + 1) * 128]
                    t = nc.tensor.matmul(
                        dst, src, id_t[:],
                        start=(j == 0), stop=(j == NJ - 1),
                        is_transpose=True,
                    )
                    if j == 0:
                        order(t, prev_pe)
                prev_pe = t

                last4 = pt[0:1, (NJ - 1) * 128:(NJ - 1) * 128 + 4]
                ft_dt = F32R if USE_F32R_MM else F32
                ft = ft_pool.tile([KC, NJ * 128], ft_dt, name=f"ft{c}",
                                  tag=f"ft{c}", bufs=2)
                if c % 2 == 0:
                    rl = nc.vector.tensor_copy(rscr[0:1, 0:4], last4)
                    if prev_dve is not None:
                        order(rl, prev_dve)
                    cp = nc.vector.tensor_copy(ft[:], pt[:])
                    prev_dve = cp
                else:
                    rl = nc.scalar.activation(
                        rscr_act[0:1, 0:4], last4,
                        mybir.ActivationFunctionType.Copy)
                    if prev_act is not None:
                        order(rl, prev_act)
                    cp = nc.scalar.activation(
                        ft[:], pt[:], mybir.ActivationFunctionType.Copy)
                    prev_act = cp
                order(cp, rl)
                fts.append(ft)

            for c in range(NCHUNK - 3, NCHUNK):
                emit_mm(c)

            ot = ot_pool.tile([S, D], F32, name="ot", tag=f"ot{b}", bufs=1)
            oc = nc.scalar.activation(ot[:], po[:],
                                      mybir.ActivationFunctionType.Copy)
            if prev_act is not None:
                order(oc, prev_act)
            prev_act = oc
            ots.append(ot)

        flush_out(B_LOC - 2)
        flush_out(B_LOC - 1)

    if SPLIT_DRAIN:
        _split_drain_waits(nc)
    return nc


def get_program_fb():
    if "nc_fb" not in _CACHE:
        _CACHE["nc_fb"] = _build_program_fb()
    return _CACHE["nc_fb"]


def _kernel_fb(feats, masks, _trace=False, _tmpdir=None):
    nc = get_program_fb()
    feats = np.ascontiguousarray(np.asarray(feats, dtype=np.float32))
    masks = np.asarray(masks, dtype=np.float32)
    masksT = np.ascontiguousarray(masks.reshape(S, HW).T)
    fr = feats.reshape(N_CORES, B_LOC, D, HW)
    in_maps = [{"feats": fr[i], "masksT": masksT} for i in range(N_CORES)]
    res = run_bass_kernel_spmd(
        nc, in_maps, core_ids=list(range(N_CORES)),
        trace=_trace, tmpdir=_tmpdir,
    )
    out = np.concatenate([r["out"] for r in res.results], axis=0)
    if _trace:
        _CACHE["last_results"] = res
    return out
